# revision 1
# baseline (speedup 1.0000x reference)
"""Trainium2 Bass kernel for GQA sliding-window causal attention.

Problem: B=2, S=2048, H=32 q-heads, KVH=8 kv-heads, D=128,
sliding window 1024, causal, scale 1/sqrt(128). f32 I/O.

Sharding (8 cores, pure tensor parallel, no collectives): core c gets
kv-head c and its query-head group [4c, 4c+4). Each core computes full
attention for its 4 q-heads over both batch elements; host concatenates
along the head dim.

Per-core algorithm (banded, no online softmax needed since scores are
O(1) and exp never overflows):
  - Q and K live in SBUF transposed: [d=128 partitions, s free].
  - Scores computed transposed, ST[k, q] = (KT_j).T-contracted-with-QT,
    per (512-wide q-block, 128-wide k-tile) over the causal+window band.
  - P = exp(SCALE * ST) on ScalarE (scale folded into the activation),
    written as bf16 to SBUF.
  - Causal-diagonal and window-edge tiles are masked AFTER exp by
    multiplying with 0/1 bf16 mask tiles on VectorE (exact zeros).
  - PV: out[q, 0:129] += PT_slice.T @ V'_j where V' has a ones column
    appended -> col 128 accumulates the softmax denominator for free.
  - Normalize: out = psum[:, :128] * reciprocal(psum[:, 128]) on DVE.
All matmuls bf16 with f32 PSUM accumulation; softmax math in f32.
"""

import numpy as np
import ml_dtypes

B = 2
S = 2048
H = 32
KVH = 8
D = 128
HQ = H // KVH  # q heads per core = 4
W = 1024  # sliding window
SCALE = 0.08838834764831845
N_CORES = 8
BS = B * S  # 4096
NT = S // 128  # 16 k-tiles / q-tiles per sequence
NG = S // 512  # 4 q-blocks per sequence
VW = D + 1  # 129: V width with ones column

_BF16 = ml_dtypes.bfloat16

_CACHE = {}


def _build_nc(reps=1, loop_reps=0, opts=None):
    """Build + compile the single-core Bass/Tile program (SPMD across 8).

    reps > 1 unrolls the whole computation inside one NEFF; loop_reps > 0
    instead wraps the body in a hardware For_i loop. Both are used only
    for timing (per-rep delta isolates kernel time from dispatch
    overhead; the grader runs the default reps=1 single-shot program).
    opts: dict of ablation/tuning switches (see _body_once).
    """
    from contextlib import ExitStack

    import concourse.bass as bass
    import concourse.tile as tile
    from concourse import bacc, mybir

    opts = dict(opts or {})
    fp32 = mybir.dt.float32
    bf16 = mybir.dt.bfloat16

    nc = bacc.Bacc("TRN2", target_bir_lowering=False, debug=False,
                   num_devices=N_CORES)

    qt_d = nc.dram_tensor("qt", [HQ, D, BS], bf16, kind="ExternalInput").ap()
    kt_d = nc.dram_tensor("kt", [D, BS], bf16, kind="ExternalInput").ap()
    vv_d = nc.dram_tensor("vv", [B, 128, NT * VW], bf16, kind="ExternalInput").ap()
    mk_d = nc.dram_tensor("mk", [128, 256], bf16, kind="ExternalInput").ap()
    out_d = nc.dram_tensor("out", [HQ, B, S, D], fp32, kind="ExternalOutput").ap()

    st_bufs = opts.get("st_bufs", 3)
    acc_bufs = opts.get("acc_bufs", 5)
    pt_bufs = opts.get("pt_bufs", 3)

    with tile.TileContext(nc) as tc, ExitStack() as ctx:
        mask_pool = ctx.enter_context(tc.tile_pool(name="mask", bufs=1))
        kt_pool = ctx.enter_context(tc.tile_pool(name="ktp", bufs=2))
        vv_pool = ctx.enter_context(tc.tile_pool(name="vvp", bufs=2))
        qt_pool = ctx.enter_context(tc.tile_pool(name="qtp", bufs=2))
        pt_pool = ctx.enter_context(tc.tile_pool(name="ptp", bufs=pt_bufs))
        osb_pool = ctx.enter_context(tc.tile_pool(name="osb", bufs=3))
        rec_pool = ctx.enter_context(tc.tile_pool(name="rec", bufs=3))
        st_pool = ctx.enter_context(
            tc.tile_pool(name="stp", bufs=st_bufs, space="PSUM"))
        acc_pool = ctx.enter_context(
            tc.tile_pool(name="accp", bufs=acc_bufs, space="PSUM"))

        masks = mask_pool.tile([128, 256], bf16)
        nc.sync.dma_start(masks[:], mk_d[:])

        const_pt = None
        if opts.get("no_exp"):
            const_pt = mask_pool.tile([128, 512], bf16, name="const_pt")
            nc.vector.memset(const_pt[:], 0.25)

        pools = (kt_pool, vv_pool, qt_pool, pt_pool, osb_pool, rec_pool,
                 st_pool, acc_pool)
        if loop_reps:
            with tc.For_i(0, loop_reps, 1,
                          hint_engines=tuple(nc.engines)) as _i:
                _body_once(nc, tc, mybir, masks, *pools,
                           qt_d, kt_d, vv_d, out_d, opts, const_pt)
        else:
            for _rep in range(reps):
                _body_once(nc, tc, mybir, masks, *pools,
                           qt_d, kt_d, vv_d, out_d, opts, const_pt)

    nc.compile()
    return nc


def _body_once(nc, tc, mybir, masks, kt_pool, vv_pool, qt_pool, pt_pool,
               osb_pool, rec_pool, st_pool, acc_pool, qt_d, kt_d, vv_d,
               out_d, opts=None, const_pt=None):
    opts = opts or {}
    no_st = opts.get("no_st", False)
    no_exp = opts.get("no_exp", False)
    no_masks = opts.get("no_masks", False)
    no_pv = opts.get("no_pv", False)
    fp32 = mybir.dt.float32
    bf16 = mybir.dt.bfloat16
    if True:
        for b in range(B):
            ktt = kt_pool.tile([128, S], bf16)
            nc.sync.dma_start(ktt[:], kt_d[:, b * S:(b + 1) * S])
            vvt = vv_pool.tile([128, NT * VW], bf16)
            nc.sync.dma_start(vvt[:], vv_d[b])
            for h in range(HQ):
                qtt = qt_pool.tile([128, S], bf16)
                nc.sync.dma_start(qtt[:], qt_d[h, :, b * S:(b + 1) * S])
                for g in range(NG):
                    q0 = 512 * g
                    # one PSUM bank per q-tile accumulator (a bank allows
                    # only one pending accumulation group at a time)
                    acc = [acc_pool.tile([128, VW], fp32, tag="acc",
                                         name=f"acc_{b}_{h}_{g}_{s_}")
                           for s_ in range(4)]
                    for j in range(max(0, 4 * g - 8), 4 * g + 4):
                        qv = max(q0, 128 * j)
                        qe = min(q0 + 512, 128 * j + 128 + W)
                        n = qe - qv
                        st = st_pool.tile([128, 512], fp32)
                        if not no_st:
                            nc.tensor.matmul(
                                st[:, :n],
                                ktt[:, 128 * j:128 * j + 128],
                                qtt[:, qv:qe],
                                start=True, stop=True,
                            )
                        if no_exp:
                            pt = const_pt
                        else:
                            pt = pt_pool.tile([128, 512], bf16)
                            nc.scalar.activation(
                                pt[:, :n], st[:, :n],
                                mybir.ActivationFunctionType.Exp, scale=SCALE,
                            )
                        if not no_masks and not no_exp:
                            if j >= 4 * g:
                                # causal diagonal tile: always first 128 cols
                                nc.vector.tensor_mul(
                                    pt[:, 0:128], pt[:, 0:128],
                                    masks[:, 0:128])
                            if qe == 128 * j + 128 + W:
                                # window edge tile: last 128 cols
                                nc.vector.tensor_mul(
                                    pt[:, n - 128:n], pt[:, n - 128:n],
                                    masks[:, 128:256])
                        for i in range(max(4 * g, j), min(4 * g + 3, j + 8) + 1):
                            s_ = i - 4 * g
                            off = 128 * i - qv
                            if not no_pv:
                                nc.tensor.matmul(
                                    acc[s_][:, :],
                                    pt[:, off:off + 128],
                                    vvt[:, VW * j:VW * j + VW],
                                    start=(j == max(0, i - 8)), stop=(j == i),
                                )
                            if j == i:
                                src = acc[s_]
                                rec = rec_pool.tile([128, 1], fp32)
                                nc.vector.reciprocal(rec[:], src[:, 128:129])
                                ot = osb_pool.tile([128, 128], fp32)
                                nc.vector.tensor_scalar_mul(
                                    ot[:], src[:, 0:128], rec[:])
                                nc.sync.dma_start(
                                    out_d[h, b, 128 * i:128 * i + 128, :],
                                    ot[:])


def _mask_np():
    """[128, 256] bf16: cols 0:128 diag keep r<=c; cols 128:256 edge keep c<r."""
    r = np.arange(128)[:, None]
    c = np.arange(128)[None, :]
    diag = (r <= c).astype(np.float32)
    edge = (c < r).astype(np.float32)
    return np.concatenate([diag, edge], axis=1).astype(_BF16)


def _prep_in_maps(query, key, value):
    q = np.asarray(query, dtype=np.float32).reshape(B, S, H, D)
    k = np.asarray(key, dtype=np.float32).reshape(B, S, KVH, D)
    v = np.asarray(value, dtype=np.float32).reshape(B, S, KVH, D)

    # [H, D, B*S] / [KVH, D, B*S]
    qt_all = np.ascontiguousarray(q.transpose(2, 3, 0, 1).reshape(H, D, BS)).astype(_BF16)
    kt_all = np.ascontiguousarray(k.transpose(2, 3, 0, 1).reshape(KVH, D, BS)).astype(_BF16)

    # V with ones column, packed [KVH, B, 128p, NT*VW] so that
    # vv[c, b, p, t*VW + d] = V'[b, 128t + p, c, d]
    vpad = np.concatenate([v, np.ones((B, S, KVH, 1), np.float32)], axis=3)
    vv_all = np.ascontiguousarray(
        vpad.reshape(B, NT, 128, KVH, VW).transpose(3, 0, 2, 1, 4)
        .reshape(KVH, B, 128, NT * VW)).astype(_BF16)

    mk = _mask_np()
    return [
        {
            "qt": np.ascontiguousarray(qt_all[HQ * c:HQ * c + HQ]),
            "kt": np.ascontiguousarray(kt_all[c]),
            "vv": np.ascontiguousarray(vv_all[c]),
            "mk": mk,
        }
        for c in range(N_CORES)
    ]


def _assemble(results):
    # results[c]["out"]: [HQ, B, S, D] -> full [B, S, H*D]
    o = np.stack([np.asarray(results[c]["out"], dtype=np.float32)
                  for c in range(N_CORES)])  # [8, HQ, B, S, D]
    return np.ascontiguousarray(
        o.transpose(2, 3, 0, 1, 4).reshape(B, S, H * D))


def kernel(query, key, value):
    from concourse import bass_utils

    if "nc" not in _CACHE:
        _CACHE["nc"] = _build_nc()
    nc = _CACHE["nc"]
    in_maps = _prep_in_maps(query, key, value)
    res = bass_utils.run_bass_kernel_spmd(
        nc, in_maps, core_ids=list(range(N_CORES)))
    return _assemble(res.results)

